# revision 32
# baseline (speedup 1.0000x reference)
"""Multi-head attention (B=2, S=2048, D=1024, H=16, Dh=64) on 8 trn2 cores.

Sharding: data-parallel over batch (2) x tensor-parallel over heads (4 groups
of 4 heads). Each core computes, for its batch b and head group hg:
  qT/kT = (W x^T) in [j, s] layout, v in [t, d] layout (all bf16),
  scoresT[t, s] = kT^T q / 8 per head (row-packed K=64 matmul pairs),
  expST = exp(scoresT) on ACT (fp32 psum in, bf16 out),
  ctxT[d, s] + softmax denominator via a ones-column appended to v (M=65),
  out_partial[s, :] = ctxT^T Wo_shard^T + bo (fp32 out).
Host sums the 4 head-group partials per batch.

Emission order interleaves qT projection s-blocks and the output projection
with the attention chunks so the PE fills the gaps of the ACT-bound phase.
"""

import ml_dtypes
import numpy as np

import concourse.bacc as bacc
import concourse.mybir as mybir
import concourse.tile as tile
from concourse.bass_utils import run_bass_kernel_spmd

F32 = mybir.dt.float32
BF16 = mybir.dt.bfloat16

S = 2048
DM = 1024
JL = 256  # local projection width = 4 heads * 64
HL = 4
DH = 64
P = 128
NK = DM // P
NJT = JL // P
NSB = S // 512
NTT = S // P
SCALE = 1.0 / np.sqrt(DH)

_CACHE = {}


def build_nc(zero_bias=False):
    nc = bacc.Bacc("TRN2", target_bir_lowering=False, debug=False, num_devices=8)

    QT = nc.declare_dram_parameter("QT", [DM, S], BF16, isOutput=False)
    KT = nc.declare_dram_parameter("KT", [DM, S], BF16, isOutput=False)
    VT = nc.declare_dram_parameter("VT", [DM, S], BF16, isOutput=False)
    WQT = nc.declare_dram_parameter("WQT", [DM, JL], BF16, isOutput=False)
    WKT = nc.declare_dram_parameter("WKT", [DM, JL], BF16, isOutput=False)
    WVT = nc.declare_dram_parameter("WVT", [DM, JL], BF16, isOutput=False)
    WOT = nc.declare_dram_parameter("WOT", [JL, DM], BF16, isOutput=False)
    BQ = nc.declare_dram_parameter("BQ", [JL], F32, isOutput=False)
    BK = nc.declare_dram_parameter("BK", [JL], F32, isOutput=False)
    BV = nc.declare_dram_parameter("BV", [JL], F32, isOutput=False)
    BO = nc.declare_dram_parameter("BO", [DM], F32, isOutput=False)
    OUT = nc.declare_dram_parameter("OUT", [S, DM], F32, isOutput=True)

    with tile.TileContext(nc) as tc:
        with (
            tc.tile_pool(name="singles", bufs=1) as singles,
            tc.tile_pool(name="spsum", bufs=2, space="PSUM") as spool,
            tc.tile_pool(name="cpsum", bufs=4, space="PSUM") as cpool,
            tc.tile_pool(name="xin", bufs=3) as xpool,
            tc.tile_pool(name="exps", bufs=4) as epool,
            tc.tile_pool(name="outs", bufs=2) as outpool,
        ):
            wqt = singles.tile([P, NK, JL], BF16)
            wkt = singles.tile([P, NK, JL], BF16)
            wvt = singles.tile([P, NK, JL], BF16)
            wot = singles.tile([P, NJT, DM], BF16)
            bq_sb = singles.tile([P, NJT], F32)
            bk_sb = singles.tile([P, NJT], F32)
            bvb = singles.tile([P, JL], F32)
            bob = singles.tile([P, DM], F32)
            qt_sb = singles.tile([P, NJT, S], BF16)
            kt_sb = singles.tile([P, NJT, S], BF16)
            vaug = singles.tile([P, NTT, HL, DH + 1], BF16)
            kxin = singles.tile([P, NSB - 1, NK, 512], BF16)
            ctxT = singles.tile([P, NJT, S], BF16)
            # head h's denominator at partition h*32 (legal engine bases)
            den_sb = singles.tile([P, NSB, 512], F32)
            rec_sb = singles.tile([P, NSB, 512], BF16)
            sel = [singles.tile([P, P], BF16, name=f"sel{jt}") for jt in range(NJT)]

            nc.vector.memset(den_sb, 1.0)
            for jt in range(NJT):
                nc.vector.memset(sel[jt], 0.0)
                for h2 in range(2):
                    r = (jt * 2 + h2) * 32
                    nc.vector.memset(
                        sel[jt][r : r + 1, h2 * DH : (h2 + 1) * DH], 1.0
                    )
            nc.vector.memset(vaug[:, :, :, DH : DH + 1], 1.0)


            def proj_kq_fillers(
                dst, w_sb, b_sb, src, tb, pfx, jts=(0, 1),
                store=None, do_dma=True, split=False,
            ):
                """Return emit-closures: first DMAs, then per-(jt,ki) matmuls,
                then the bias drain. PSUM accumulation groups tolerate other
                matmuls interleaved between members (state lives in the bank)."""
                state = {}

                def dmas():
                    if store is None:
                        xin = xpool.tile(
                            [P, NK, 512], BF16, tag="xin", name=f"xin{pfx}{tb}"
                        )
                        state["xin"] = xin
                    else:
                        state["xin"] = store
                    for ki in range(NK):
                        eng = nc.scalar if (split and ki % 2) else nc.sync
                        eng.dma_start(
                            out=state["xin"][:, ki, :],
                            in_=src[ki * P : (ki + 1) * P, tb * 512 : (tb + 1) * 512],
                        )

                def mk_mm(jt, ki):
                    def emit():
                        if ki == 0:
                            state[jt] = cpool.tile(
                                [P, 512], F32, tag="cb", name=f"pp{pfx}{tb}_{jt}"
                            )
                        nc.tensor.matmul(
                            state[jt],
                            w_sb[:, ki, jt * P : (jt + 1) * P],
                            state["xin"][:, ki, :],
                            start=(ki == 0),
                            stop=(ki == NK - 1),
                        )
                        if ki == NK - 1:
                            dslice = dst[:, jt, tb * 512 : (tb + 1) * 512]
                            if zero_bias:
                                nc.vector.tensor_copy(dslice, state[jt])
                            else:
                                nc.vector.tensor_scalar_add(
                                    dslice, state[jt], b_sb[:, jt : jt + 1]
                                )
                    return emit

                def dues(base):
                    out = []
                    if do_dma:
                        out.append((max(0, base - 6), dmas))
                    else:
                        state["xin"] = store
                    for j_i, jt in enumerate(jts):
                        for ki in range(NK):
                            out.append(
                                (max(0, base - 4 + ki // 2 + j_i), mk_mm(jt, ki))
                            )
                    return out

                return dues

            def proj_kq_due(
                dst, w_sb, b_sb, src, tb, pfx, jts=(0, 1), base=99, **kw
            ):
                return proj_kq_fillers(dst, w_sb, b_sb, src, tb, pfx, jts, **kw)(base)

            def proj_kq(dst, w_sb, b_sb, src, tb, pfx, jts=(0, 1), **kw):
                for _, f in proj_kq_due(dst, w_sb, b_sb, src, tb, pfx, jts, 0, **kw):
                    f()

            def proj_v_fillers(tb):
                state = {}

                def dmas():
                    xin = xpool.tile([P, NK, 512], BF16, tag="xin", name=f"xv{tb}")
                    state["xin"] = xin
                    for ki in range(NK):
                        eng = nc.scalar if (tb == 0 and ki % 2) else nc.sync
                        eng.dma_start(
                            out=xin[:, ki, :],
                            in_=VT[ki * P : (ki + 1) * P, tb * 512 : (tb + 1) * 512],
                        )

                def mk_mm(tl, ki):
                    def emit():
                        if tl % 2 == 0 and ki == 0:
                            state[tl // 2] = cpool.tile(
                                [P, 512], F32, tag="cb", name=f"pv{tb}_{tl // 2}"
                            )
                        nc.tensor.matmul(
                            state[tl // 2][:, (tl % 2) * JL : (tl % 2 + 1) * JL],
                            state["xin"][:, ki, tl * P : (tl + 1) * P],
                            wvt[:, ki, :],
                            start=(ki == 0),
                            stop=(ki == NK - 1),
                        )
                        if ki == NK - 1:
                            tt = tb * 4 + tl
                            pvv = state[tl // 2][
                                :, (tl % 2) * JL : (tl % 2 + 1) * JL
                            ].rearrange("p (h d) -> p h d", h=HL)
                            if zero_bias:
                                nc.vector.tensor_copy(vaug[:, tt, :, 0:DH], pvv)
                            else:
                                nc.vector.tensor_add(
                                    vaug[:, tt, :, 0:DH],
                                    pvv,
                                    bvb.rearrange("p (h d) -> p h d", h=HL),
                                )
                    return emit

                def dues():
                    out = [(max(0, tb * 4 - 6), dmas)]
                    for tl in range(4):
                        # one uniform due per (tl) group: groups sharing a psum
                        # bank must not interleave their accumulation windows
                        need = tb * 4 + tl
                        for ki in range(NK):
                            out.append((max(0, need - 2), mk_mm(tl, ki)))
                    return out

                return dues

            def proj_v_due(tb):
                return proj_v_fillers(tb)()

            def proj_v(tb):
                for _, f in proj_v_due(tb):
                    f()

            def chunk(sb, jt, fillers=(), rate=2):
                # fillers: list of (due_tt, fn); all fillers with due <= tt are
                # emitted at the end of iteration tt (plus `rate` extras).
                fillers = sorted(fillers, key=lambda df: df[0])
                cps = [
                    cpool.tile(
                        [DH + 1, 512], F32, tag="cb", name=f"cps{sb}_{jt}_{i}"
                    )
                    for i in range(2)
                ]
                for tt in range(NTT):
                    sps = spool.tile([P, 1024], F32, tag="sps", name=f"sps{sb}_{jt}_{tt}")
                    for h2 in range(2):
                        ho = h2 * DH
                        nc.tensor.matmul(
                            sps[:, h2 * 512 : (h2 + 1) * 512],
                            kt_sb[ho : ho + DH, jt, tt * P : (tt + 1) * P],
                            qt_sb[ho : ho + DH, jt, sb * 512 : (sb + 1) * 512],
                            start=True,
                            stop=True,
                        )
                    ex = epool.tile([P, 1024], BF16, tag="ex", name=f"ex{sb}_{jt}_{tt}")
                    nc.scalar.activation(
                        out=ex,
                        in_=sps,
                        func=mybir.ActivationFunctionType.Exp,
                        scale=float(SCALE),
                    )
                    for h2 in range(2):
                        h = jt * 2 + h2
                        nc.tensor.matmul(
                            cps[h2],
                            vaug[:, tt, h, :],
                            ex[:, h2 * 512 : (h2 + 1) * 512],
                            start=(tt == 0),
                            stop=(tt == NTT - 1),
                        )
                    if tt == NTT - 1:
                        while fillers:
                            fillers.pop(0)[1]()
                    else:
                        while fillers and fillers[0][0] <= tt:
                            fillers.pop(0)[1]()
                        for _ in range(rate):
                            if fillers and fillers[0][0] <= tt + 6:
                                fillers.pop(0)[1]()
                            else:
                                break
                for h2 in range(2):
                    h = jt * 2 + h2
                    nc.vector.tensor_copy(
                        ctxT[h2 * DH : (h2 + 1) * DH, jt, sb * 512 : (sb + 1) * 512],
                        cps[h2][0:DH, :],
                    )
                    nc.vector.tensor_copy(
                        den_sb[h * 32 : h * 32 + 1, sb, :],
                        cps[h2][DH : DH + 1, :],
                    )

            def norm_fillers(sb, jt2):
                # one jt half: reciprocal of this pair's two denominator rows
                # (partitions jt2*64..+64), then broadcast + scale its ctxT half
                pr = slice(jt2 * DH, (jt2 + 1) * DH)

                def recip():
                    with nc.allow_low_precision(reason="softmax scale is bf16 anyway"):
                        nc.vector.reciprocal(rec_sb[pr, sb, :], den_sb[pr, sb, :])

                def emit():
                    rb = cpool.tile([P, 512], F32, tag="cb", name=f"rb{sb}_{jt2}")
                    nc.tensor.matmul(
                        rb,
                        sel[jt2][pr, :],
                        rec_sb[pr, sb, :],
                        start=True,
                        stop=True,
                    )
                    rbb = epool.tile(
                        [P, 512], BF16, tag="rbb", bufs=2, name=f"rbb{sb}_{jt2}"
                    )
                    nc.vector.tensor_copy(rbb, rb)
                    seg = ctxT[:, jt2, sb * 512 : (sb + 1) * 512]
                    nc.vector.tensor_mul(seg, seg, rbb)

                return [(0, recip), (5, emit)]

            def norm(sb, jt2):
                for _, f in norm_fillers(sb, jt2):
                    f()

            def outproj_fillers(sb):
                def mk(stl):
                    def emit():
                        st = sb * 4 + stl
                        po = [
                            cpool.tile([P, 512], F32, tag="cb", name=f"po{st}_{i}")
                            for i in range(2)
                        ]
                        for nb in range(2):
                            for jt2 in range(NJT):
                                nc.tensor.matmul(
                                    po[nb],
                                    ctxT[:, jt2, st * P : (st + 1) * P],
                                    wot[:, jt2, nb * 512 : (nb + 1) * 512],
                                    start=(jt2 == 0),
                                    stop=(jt2 == NJT - 1),
                                )
                        ot = outpool.tile([P, DM], F32, tag="ot", name=f"ot{st}")
                        for nb in range(2):
                            osl = ot[:, nb * 512 : (nb + 1) * 512]
                            if zero_bias:
                                nc.vector.tensor_copy(osl, po[nb])
                            else:
                                nc.vector.tensor_add(
                                    osl, po[nb], bob[:, nb * 512 : (nb + 1) * 512]
                                )
                        eng = nc.gpsimd if st % 2 == 0 else nc.sync
                        eng.dma_start(out=OUT[st * P : (st + 1) * P, :], in_=ot)
                    return emit

                return [(8 + 2 * stl, mk(stl)) for stl in range(4)]

            def outproj(sb):
                for _, f in outproj_fillers(sb):
                    f()

            # ---- emission (order == static per-engine priority) ----
            # weights on sync just before first use (ACT queue stays exp-only)
            nc.sync.dma_start(out=wkt, in_=WKT.rearrange("(k p) j -> p k j", p=P))
            nc.sync.dma_start(out=bk_sb, in_=BK.rearrange("(n p) -> p n", p=P))
            proj_kq(kt_sb, wkt, bk_sb, KT, 0, "k", split=True)
            nc.sync.dma_start(out=bq_sb, in_=BQ.rearrange("(n p) -> p n", p=P))
            nc.sync.dma_start(out=wqt, in_=WQT.rearrange("(k p) j -> p k j", p=P))
            proj_kq(qt_sb, wqt, bq_sb, QT, 0, "q", split=True)
            nc.sync.dma_start(
                out=bvb, in_=BV.reshape([1, JL])[:].to_broadcast((P, JL))
            )
            nc.sync.dma_start(out=wvt, in_=WVT.rearrange("(k p) j -> p k j", p=P))
            proj_v(0)

            # chunk(0,0) carries the remaining kT-jt0 and v projections as
            # due-dated fillers (tile tt produced before iteration tt reads it)
            f0 = []
            for tb in range(1, NSB):
                f0 += proj_kq_due(
                    kt_sb, wkt, bk_sb, KT, tb, "k", (0,), tb * 4,
                    store=kxin[:, tb - 1],
                )
                f0 += proj_v_due(tb)
            chunk(0, 0, fillers=f0, rate=3)

            # chunk(0,1) carries kT-jt1 tb1..3 (due-dated) + qT sb1
            f1 = []
            for tb in range(1, NSB):
                f1 += proj_kq_due(
                    kt_sb, wkt, bk_sb, KT, tb, "kb", (1,), tb * 4,
                    store=kxin[:, tb - 1], do_dma=False,
                )
            f1 += proj_kq_due(qt_sb, wqt, bq_sb, QT, 1, "q", (0, 1), 8)
            f1 += [(d + 6, f) for d, f in norm_fillers(0, 0)]
            chunk(0, 1, fillers=f1, rate=3)

            nc.sync.dma_start(
                out=bob, in_=BO.reshape([1, DM])[:].to_broadcast((P, DM))
            )
            nc.sync.dma_start(out=wot, in_=WOT.rearrange("(n p) m -> p n m", p=P))
            for sb in range(1, NSB):
                f_a = norm_fillers(sb - 1, 1) + outproj_fillers(sb - 1)
                chunk(sb, 0, fillers=f_a)
                f_b = list(norm_fillers(sb, 0))
                if sb < NSB - 1:
                    f_b += proj_kq_due(qt_sb, wqt, bq_sb, QT, sb + 1, "q", (0, 1), 4)
                chunk(sb, 1, fillers=f_b)
            norm(NSB - 1, 1)
            outproj(NSB - 1)

    nc.compile()
    return nc


def _shard_inputs(Q, K, V, Wq, bq, Wk, bk, Wv, bv, Wo, bo):
    in_maps = []
    xt = {}
    for b in range(2):
        xt[b] = tuple(
            np.ascontiguousarray(np.asarray(a[b], dtype=np.float32).T).astype(
                ml_dtypes.bfloat16
            )
            for a in (Q, K, V)
        )
    for c in range(8):
        b, hg = c // 4, c % 4
        sl = slice(hg * JL, (hg + 1) * JL)
        qt, kt, vt = xt[b]
        in_maps.append(
            {
                "QT": qt,
                "KT": kt,
                "VT": vt,
                "WQT": np.ascontiguousarray(
                    np.asarray(Wq, np.float32)[sl, :].T
                ).astype(ml_dtypes.bfloat16),
                "WKT": np.ascontiguousarray(
                    np.asarray(Wk, np.float32)[sl, :].T
                ).astype(ml_dtypes.bfloat16),
                "WVT": np.ascontiguousarray(
                    np.asarray(Wv, np.float32)[sl, :].T
                ).astype(ml_dtypes.bfloat16),
                "WOT": np.ascontiguousarray(
                    np.asarray(Wo, np.float32)[:, sl].T
                ).astype(ml_dtypes.bfloat16),
                "BQ": np.ascontiguousarray(np.asarray(bq, np.float32)[sl]),
                "BK": np.ascontiguousarray(np.asarray(bk, np.float32)[sl]),
                "BV": np.ascontiguousarray(np.asarray(bv, np.float32)[sl]),
                "BO": (
                    np.ascontiguousarray(np.asarray(bo, np.float32))
                    if hg == 0
                    else np.zeros(DM, np.float32)
                ),
            }
        )
    return in_maps


def kernel(Q, K, V, Wq, bq, Wk, bk, Wv, bv, Wo, bo):
    zb = all(
        not np.any(np.asarray(b, np.float32)) for b in (bq, bk, bv, bo)
    )
    key = ("nc", zb)
    if key not in _CACHE:
        _CACHE[key] = build_nc(zero_bias=zb)
    nc = _CACHE[key]
    in_maps = _shard_inputs(Q, K, V, Wq, bq, Wk, bk, Wv, bv, Wo, bo)
    res = run_bass_kernel_spmd(nc, in_maps, list(range(8)))
    out = np.zeros((2, S, DM), np.float32)
    for c in range(8):
        out[c // 4] += res.results[c]["OUT"]
    return out


# revision 33
# speedup vs baseline: 1.0818x; 1.0818x over previous
"""Multi-head attention (B=2, S=2048, D=1024, H=16, Dh=64) on 8 trn2 cores.

Sharding: data-parallel over batch (2) x tensor-parallel over heads (4 groups
of 4 heads). Each core computes, for its batch b and head group hg:
  qT/kT = (W x^T) in [j, s] layout, v in [t, d] layout (all bf16),
  scoresT[t, s] = kT^T q / 8 per head (row-packed K=64 matmul pairs),
  expST = exp(scoresT) on ACT (fp32 psum in, bf16 out),
  ctxT[d, s] + softmax denominator via a ones-column appended to v (M=65),
  out_partial[s, :] = ctxT^T Wo_shard^T + bo (fp32 out).
Host sums the 4 head-group partials per batch.

Emission order interleaves qT projection s-blocks and the output projection
with the attention chunks so the PE fills the gaps of the ACT-bound phase.
"""

import ml_dtypes
import numpy as np

import concourse.bacc as bacc
import concourse.mybir as mybir
import concourse.tile as tile
from concourse.bass_utils import run_bass_kernel_spmd

F32 = mybir.dt.float32
BF16 = mybir.dt.bfloat16

S = 2048
DM = 1024
JL = 256  # local projection width = 4 heads * 64
HL = 4
DH = 64
P = 128
NK = DM // P
NJT = JL // P
NSB = S // 512
NTT = S // P
SCALE = 1.0 / np.sqrt(DH)

_CACHE = {}

_GAT_ORIG = bacc.get_activation_tables


def _gat_pinned(arch):
    t = _GAT_ORIG(arch)
    keep = "natural_log_exp_and_others"
    E, L = mybir.ActivationFunctionType.Exp, mybir.ActivationFunctionType.Ln
    if keep in t and E in t[keep] and L in t[keep]:
        for name, funcs in t.items():
            if name != keep:
                funcs.discard(E)
                funcs.discard(L)
    return t


bacc.get_activation_tables = _gat_pinned


def build_nc(zero_bias=False):
    nc = bacc.Bacc("TRN2", target_bir_lowering=False, debug=False, num_devices=8)

    QT = nc.declare_dram_parameter("QT", [DM, S], BF16, isOutput=False)
    KT = nc.declare_dram_parameter("KT", [DM, S], BF16, isOutput=False)
    VT = nc.declare_dram_parameter("VT", [DM, S], BF16, isOutput=False)
    WQT = nc.declare_dram_parameter("WQT", [DM, JL], BF16, isOutput=False)
    WKT = nc.declare_dram_parameter("WKT", [DM, JL], BF16, isOutput=False)
    WVT = nc.declare_dram_parameter("WVT", [DM, JL], BF16, isOutput=False)
    WOT = nc.declare_dram_parameter("WOT", [JL, DM], BF16, isOutput=False)
    BQ = nc.declare_dram_parameter("BQ", [JL], F32, isOutput=False)
    BK = nc.declare_dram_parameter("BK", [JL], F32, isOutput=False)
    BV = nc.declare_dram_parameter("BV", [JL], F32, isOutput=False)
    BO = nc.declare_dram_parameter("BO", [DM], F32, isOutput=False)
    OUT = nc.declare_dram_parameter("OUT", [S, DM], F32, isOutput=True)

    with tile.TileContext(nc) as tc:
        with (
            tc.tile_pool(name="singles", bufs=1) as singles,
            tc.tile_pool(name="spsum", bufs=2, space="PSUM") as spool,
            tc.tile_pool(name="cpsum", bufs=4, space="PSUM") as cpool,
            tc.tile_pool(name="xin", bufs=3) as xpool,
            tc.tile_pool(name="exps", bufs=4) as epool,
            tc.tile_pool(name="outs", bufs=2) as outpool,
        ):
            wqt = singles.tile([P, NK, JL], BF16)
            wkt = singles.tile([P, NK, JL], BF16)
            wvt = singles.tile([P, NK, JL], BF16)
            wot = singles.tile([P, NJT, DM], BF16)
            bq_sb = singles.tile([P, NJT], F32)
            bk_sb = singles.tile([P, NJT], F32)
            bvb = singles.tile([P, JL], F32)
            bob = singles.tile([P, DM], F32)
            qt_sb = singles.tile([P, NJT, S], BF16)
            kt_sb = singles.tile([P, NJT, S], BF16)
            vaug = singles.tile([P, NTT, HL, DH + 1], BF16)
            kxin = singles.tile([P, NSB - 1, NK, 512], BF16)
            ctxT = singles.tile([P, NJT, S], BF16)
            # head h's denominator at partition h*32 (legal engine bases)
            den_sb = singles.tile([P, NSB, 512], F32)
            rec_sb = singles.tile([P, NSB, 512], BF16)
            lntmp = singles.tile([P, 512], F32)
            sel = [singles.tile([P, P], BF16, name=f"sel{jt}") for jt in range(NJT)]

            nc.vector.memset(den_sb, 1.0)
            for jt in range(NJT):
                nc.vector.memset(sel[jt], 0.0)
                for h2 in range(2):
                    r = (jt * 2 + h2) * 32
                    nc.vector.memset(
                        sel[jt][r : r + 1, h2 * DH : (h2 + 1) * DH], 1.0
                    )
            nc.vector.memset(vaug[:, :, :, DH : DH + 1], 1.0)


            def proj_kq_fillers(
                dst, w_sb, b_sb, src, tb, pfx, jts=(0, 1),
                store=None, do_dma=True, split=False,
            ):
                """Return emit-closures: first DMAs, then per-(jt,ki) matmuls,
                then the bias drain. PSUM accumulation groups tolerate other
                matmuls interleaved between members (state lives in the bank)."""
                state = {}

                def dmas():
                    if store is None:
                        xin = xpool.tile(
                            [P, NK, 512], BF16, tag="xin", name=f"xin{pfx}{tb}"
                        )
                        state["xin"] = xin
                    else:
                        state["xin"] = store
                    for ki in range(NK):
                        eng = nc.scalar if (split and ki % 2) else nc.sync
                        eng.dma_start(
                            out=state["xin"][:, ki, :],
                            in_=src[ki * P : (ki + 1) * P, tb * 512 : (tb + 1) * 512],
                        )

                def mk_mm(jt, ki):
                    def emit():
                        if ki == 0:
                            state[jt] = cpool.tile(
                                [P, 512], F32, tag="cb", name=f"pp{pfx}{tb}_{jt}"
                            )
                        nc.tensor.matmul(
                            state[jt],
                            w_sb[:, ki, jt * P : (jt + 1) * P],
                            state["xin"][:, ki, :],
                            start=(ki == 0),
                            stop=(ki == NK - 1),
                        )
                        if ki == NK - 1:
                            dslice = dst[:, jt, tb * 512 : (tb + 1) * 512]
                            if zero_bias:
                                nc.vector.tensor_copy(dslice, state[jt])
                            else:
                                nc.vector.tensor_scalar_add(
                                    dslice, state[jt], b_sb[:, jt : jt + 1]
                                )
                    return emit

                def dues(base):
                    out = []
                    if do_dma:
                        out.append((max(0, base - 6), dmas))
                    else:
                        state["xin"] = store
                    for j_i, jt in enumerate(jts):
                        for ki in range(NK):
                            out.append(
                                (max(0, base - 4 + ki // 2 + j_i), mk_mm(jt, ki))
                            )
                    return out

                return dues

            def proj_kq_due(
                dst, w_sb, b_sb, src, tb, pfx, jts=(0, 1), base=99, **kw
            ):
                return proj_kq_fillers(dst, w_sb, b_sb, src, tb, pfx, jts, **kw)(base)

            def proj_kq(dst, w_sb, b_sb, src, tb, pfx, jts=(0, 1), **kw):
                for _, f in proj_kq_due(dst, w_sb, b_sb, src, tb, pfx, jts, 0, **kw):
                    f()

            def proj_v_fillers(tb):
                state = {}

                def dmas():
                    xin = xpool.tile([P, NK, 512], BF16, tag="xin", name=f"xv{tb}")
                    state["xin"] = xin
                    for ki in range(NK):
                        eng = nc.scalar if (tb == 0 and ki % 2) else nc.sync
                        eng.dma_start(
                            out=xin[:, ki, :],
                            in_=VT[ki * P : (ki + 1) * P, tb * 512 : (tb + 1) * 512],
                        )

                def mk_mm(tl, ki):
                    def emit():
                        if tl % 2 == 0 and ki == 0:
                            state[tl // 2] = cpool.tile(
                                [P, 512], F32, tag="cb", name=f"pv{tb}_{tl // 2}"
                            )
                        nc.tensor.matmul(
                            state[tl // 2][:, (tl % 2) * JL : (tl % 2 + 1) * JL],
                            state["xin"][:, ki, tl * P : (tl + 1) * P],
                            wvt[:, ki, :],
                            start=(ki == 0),
                            stop=(ki == NK - 1),
                        )
                        if ki == NK - 1:
                            tt = tb * 4 + tl
                            pvv = state[tl // 2][
                                :, (tl % 2) * JL : (tl % 2 + 1) * JL
                            ].rearrange("p (h d) -> p h d", h=HL)
                            if zero_bias:
                                nc.vector.tensor_copy(vaug[:, tt, :, 0:DH], pvv)
                            else:
                                nc.vector.tensor_add(
                                    vaug[:, tt, :, 0:DH],
                                    pvv,
                                    bvb.rearrange("p (h d) -> p h d", h=HL),
                                )
                    return emit

                def dues():
                    out = [(max(0, tb * 4 - 6), dmas)]
                    for tl in range(4):
                        # one uniform due per (tl) group: groups sharing a psum
                        # bank must not interleave their accumulation windows
                        need = tb * 4 + tl
                        for ki in range(NK):
                            out.append((max(0, need - 2), mk_mm(tl, ki)))
                    return out

                return dues

            def proj_v_due(tb):
                return proj_v_fillers(tb)()

            def proj_v(tb):
                for _, f in proj_v_due(tb):
                    f()

            def chunk(sb, jt, fillers=(), rate=2):
                # fillers: list of (due_tt, fn); all fillers with due <= tt are
                # emitted at the end of iteration tt (plus `rate` extras).
                fillers = sorted(fillers, key=lambda df: df[0])
                cps = [
                    cpool.tile(
                        [DH + 1, 512], F32, tag="cb", name=f"cps{sb}_{jt}_{i}"
                    )
                    for i in range(2)
                ]
                for tt in range(NTT):
                    sps = spool.tile([P, 1024], F32, tag="sps", name=f"sps{sb}_{jt}_{tt}")
                    for h2 in range(2):
                        ho = h2 * DH
                        nc.tensor.matmul(
                            sps[:, h2 * 512 : (h2 + 1) * 512],
                            kt_sb[ho : ho + DH, jt, tt * P : (tt + 1) * P],
                            qt_sb[ho : ho + DH, jt, sb * 512 : (sb + 1) * 512],
                            start=True,
                            stop=True,
                        )
                    ex = epool.tile([P, 1024], BF16, tag="ex", name=f"ex{sb}_{jt}_{tt}")
                    nc.scalar.activation(
                        out=ex,
                        in_=sps,
                        func=mybir.ActivationFunctionType.Exp,
                        scale=float(SCALE),
                    )
                    for h2 in range(2):
                        h = jt * 2 + h2
                        nc.tensor.matmul(
                            cps[h2],
                            vaug[:, tt, h, :],
                            ex[:, h2 * 512 : (h2 + 1) * 512],
                            start=(tt == 0),
                            stop=(tt == NTT - 1),
                        )
                    if tt == NTT - 1:
                        while fillers:
                            fillers.pop(0)[1]()
                    else:
                        while fillers and fillers[0][0] <= tt:
                            fillers.pop(0)[1]()
                        for _ in range(rate):
                            if fillers and fillers[0][0] <= tt + 6:
                                fillers.pop(0)[1]()
                            else:
                                break
                for h2 in range(2):
                    h = jt * 2 + h2
                    nc.vector.tensor_copy(
                        ctxT[h2 * DH : (h2 + 1) * DH, jt, sb * 512 : (sb + 1) * 512],
                        cps[h2][0:DH, :],
                    )
                    nc.vector.tensor_copy(
                        den_sb[h * 32 : h * 32 + 1, sb, :],
                        cps[h2][DH : DH + 1, :],
                    )

            def norm_fillers(sb, jt2):
                # one jt half: reciprocal of this pair's two denominator rows
                # (partitions jt2*64..+64), then broadcast + scale its ctxT half
                pr = slice(jt2 * DH, (jt2 + 1) * DH)

                def recip():
                    # 1/d = exp(-ln d): two ACT ops; both funcs pinned to the
                    # natural_log_exp table set so no table reloads occur
                    nc.scalar.activation(
                        out=lntmp[pr, :],
                        in_=den_sb[pr, sb, :],
                        func=mybir.ActivationFunctionType.Ln,
                    )
                    nc.scalar.activation(
                        out=rec_sb[pr, sb, :],
                        in_=lntmp[pr, :],
                        func=mybir.ActivationFunctionType.Exp,
                        scale=-1.0,
                    )

                def emit():
                    rb = cpool.tile([P, 512], F32, tag="cb", name=f"rb{sb}_{jt2}")
                    nc.tensor.matmul(
                        rb,
                        sel[jt2][pr, :],
                        rec_sb[pr, sb, :],
                        start=True,
                        stop=True,
                    )
                    rbb = epool.tile(
                        [P, 512], BF16, tag="rbb", bufs=2, name=f"rbb{sb}_{jt2}"
                    )
                    nc.vector.tensor_copy(rbb, rb)
                    seg = ctxT[:, jt2, sb * 512 : (sb + 1) * 512]
                    nc.vector.tensor_mul(seg, seg, rbb)

                return [(0, recip), (5, emit)]

            def norm(sb, jt2):
                for _, f in norm_fillers(sb, jt2):
                    f()

            def outproj_fillers(sb):
                def mk(stl):
                    def emit():
                        st = sb * 4 + stl
                        po = [
                            cpool.tile([P, 512], F32, tag="cb", name=f"po{st}_{i}")
                            for i in range(2)
                        ]
                        for nb in range(2):
                            for jt2 in range(NJT):
                                nc.tensor.matmul(
                                    po[nb],
                                    ctxT[:, jt2, st * P : (st + 1) * P],
                                    wot[:, jt2, nb * 512 : (nb + 1) * 512],
                                    start=(jt2 == 0),
                                    stop=(jt2 == NJT - 1),
                                )
                        ot = outpool.tile([P, DM], F32, tag="ot", name=f"ot{st}")
                        for nb in range(2):
                            osl = ot[:, nb * 512 : (nb + 1) * 512]
                            if zero_bias:
                                nc.vector.tensor_copy(osl, po[nb])
                            else:
                                nc.vector.tensor_add(
                                    osl, po[nb], bob[:, nb * 512 : (nb + 1) * 512]
                                )
                        eng = nc.gpsimd if st % 2 == 0 else nc.sync
                        eng.dma_start(out=OUT[st * P : (st + 1) * P, :], in_=ot)
                    return emit

                return [(8 + 2 * stl, mk(stl)) for stl in range(4)]

            def outproj(sb):
                for _, f in outproj_fillers(sb):
                    f()

            # ---- emission (order == static per-engine priority) ----
            # weights on sync just before first use (ACT queue stays exp-only)
            nc.sync.dma_start(out=wkt, in_=WKT.rearrange("(k p) j -> p k j", p=P))
            nc.sync.dma_start(out=bk_sb, in_=BK.rearrange("(n p) -> p n", p=P))
            proj_kq(kt_sb, wkt, bk_sb, KT, 0, "k", split=True)
            nc.sync.dma_start(out=bq_sb, in_=BQ.rearrange("(n p) -> p n", p=P))
            nc.sync.dma_start(out=wqt, in_=WQT.rearrange("(k p) j -> p k j", p=P))
            proj_kq(qt_sb, wqt, bq_sb, QT, 0, "q", split=True)
            nc.sync.dma_start(
                out=bvb, in_=BV.reshape([1, JL])[:].to_broadcast((P, JL))
            )
            nc.sync.dma_start(out=wvt, in_=WVT.rearrange("(k p) j -> p k j", p=P))
            proj_v(0)

            # chunk(0,0) carries the remaining kT-jt0 and v projections as
            # due-dated fillers (tile tt produced before iteration tt reads it)
            f0 = []
            for tb in range(1, NSB):
                f0 += proj_kq_due(
                    kt_sb, wkt, bk_sb, KT, tb, "k", (0,), tb * 4,
                    store=kxin[:, tb - 1],
                )
                f0 += proj_v_due(tb)
            chunk(0, 0, fillers=f0, rate=3)

            # chunk(0,1) carries kT-jt1 tb1..3 (due-dated) + qT sb1
            f1 = []
            for tb in range(1, NSB):
                f1 += proj_kq_due(
                    kt_sb, wkt, bk_sb, KT, tb, "kb", (1,), tb * 4,
                    store=kxin[:, tb - 1], do_dma=False,
                )
            f1 += proj_kq_due(qt_sb, wqt, bq_sb, QT, 1, "q", (0, 1), 8)
            f1 += [(d + 6, f) for d, f in norm_fillers(0, 0)]
            chunk(0, 1, fillers=f1, rate=3)

            nc.sync.dma_start(
                out=bob, in_=BO.reshape([1, DM])[:].to_broadcast((P, DM))
            )
            nc.sync.dma_start(out=wot, in_=WOT.rearrange("(n p) m -> p n m", p=P))
            for sb in range(1, NSB):
                f_a = norm_fillers(sb - 1, 1) + outproj_fillers(sb - 1)
                chunk(sb, 0, fillers=f_a)
                f_b = list(norm_fillers(sb, 0))
                if sb < NSB - 1:
                    f_b += proj_kq_due(qt_sb, wqt, bq_sb, QT, sb + 1, "q", (0, 1), 4)
                chunk(sb, 1, fillers=f_b)
            norm(NSB - 1, 1)
            outproj(NSB - 1)

    nc.compile()
    return nc


def _shard_inputs(Q, K, V, Wq, bq, Wk, bk, Wv, bv, Wo, bo):
    in_maps = []
    xt = {}
    for b in range(2):
        xt[b] = tuple(
            np.ascontiguousarray(np.asarray(a[b], dtype=np.float32).T).astype(
                ml_dtypes.bfloat16
            )
            for a in (Q, K, V)
        )
    for c in range(8):
        b, hg = c // 4, c % 4
        sl = slice(hg * JL, (hg + 1) * JL)
        qt, kt, vt = xt[b]
        in_maps.append(
            {
                "QT": qt,
                "KT": kt,
                "VT": vt,
                "WQT": np.ascontiguousarray(
                    np.asarray(Wq, np.float32)[sl, :].T
                ).astype(ml_dtypes.bfloat16),
                "WKT": np.ascontiguousarray(
                    np.asarray(Wk, np.float32)[sl, :].T
                ).astype(ml_dtypes.bfloat16),
                "WVT": np.ascontiguousarray(
                    np.asarray(Wv, np.float32)[sl, :].T
                ).astype(ml_dtypes.bfloat16),
                "WOT": np.ascontiguousarray(
                    np.asarray(Wo, np.float32)[:, sl].T
                ).astype(ml_dtypes.bfloat16),
                "BQ": np.ascontiguousarray(np.asarray(bq, np.float32)[sl]),
                "BK": np.ascontiguousarray(np.asarray(bk, np.float32)[sl]),
                "BV": np.ascontiguousarray(np.asarray(bv, np.float32)[sl]),
                "BO": (
                    np.ascontiguousarray(np.asarray(bo, np.float32))
                    if hg == 0
                    else np.zeros(DM, np.float32)
                ),
            }
        )
    return in_maps


def kernel(Q, K, V, Wq, bq, Wk, bk, Wv, bv, Wo, bo):
    zb = all(
        not np.any(np.asarray(b, np.float32)) for b in (bq, bk, bv, bo)
    )
    key = ("nc", zb)
    if key not in _CACHE:
        _CACHE[key] = build_nc(zero_bias=zb)
    nc = _CACHE[key]
    in_maps = _shard_inputs(Q, K, V, Wq, bq, Wk, bk, Wv, bv, Wo, bo)
    res = run_bass_kernel_spmd(nc, in_maps, list(range(8)))
    out = np.zeros((2, S, DM), np.float32)
    for c in range(8):
        out[c // 4] += res.results[c]["OUT"]
    return out
